# revision 28
# baseline (speedup 1.0000x reference)
"""CrossTransformer kernel for Trainium2, data-parallel over batch across 8 cores.

Math per batch b (B=32, N=25, C=512, H=W=14, DK=DV=128):
  qq = Wqk @ Q    [128, 196]      qv = Wv @ Q     [128, 196]
  K  = Wqk @ S    [128, 4900]     V  = Wv @ S     [128, 4900]
  simT[nij, hw] = K^T @ qq        (computed directly in transposed layout)
  E = exp(simT)                   (no max subtraction; |sim| <~ 60 is safe in fp32)
  ctx_raw[hw, dv] = sum_nij E[nij, hw]^T @ V^T;  den[hw] = sum_nij E[nij, hw]
  ctx = ctx_raw / den
  partial += sum((qv^T - ctx)^2)
Output per core: scalar partial sum over its 4 batches; host sums and divides by H*W.

Pipeline: S streams in n-aligned [128, 980] fp32 DMA groups (784B descriptors,
the DMA-bandwidth floor). Attention work (V transpose, sim, exp, PV accumulate)
is emitted chunk-by-chunk as soon as projection tiles cover it, so every engine
runs concurrently with the DMA stream and the post-DMA tail is tiny.
"""

import os
import sys

sys.path.insert(0, "/opt/trn_rl_repo")

import numpy as np

ILABELS = {}


def _lab(inst, label):
    try:
        ILABELS[inst.ins.name] = label
    except Exception:
        pass
    return inst

import concourse.bass as bass
import concourse.bacc as bacc
import concourse.mybir as mybir
import concourse.tile as tile
from concourse.bass_utils import run_bass_kernel_spmd
from concourse.masks import make_identity

F32 = mybir.dt.float32
F32R = mybir.dt.float32r
BF16 = mybir.dt.bfloat16

B_PER_CORE = 4
N_SUP = 25
C = 512
HW = 196
NIJ = N_SUP * HW  # 4900
DK = 128
CCH = C // 128            # 4 c-chunks
GN = 5                    # support images per DMA group
GW = GN * HW              # 980 nij per group
NG = NIJ // GW            # 5 groups per batch
FT = 490                  # matmul tile width (2 per group, fits one PSUM bank)
NCH = (NIJ + 127) // 128  # 39 nij chunks of <=128
NPAIR = (NCH + 1) // 2    # 20 sim/exp pairs (19 full + 1 solo)


def build_bass():
    nc = bacc.Bacc(
        "TRN2", target_bir_lowering=False, debug=False, enable_asserts=False
    )
    q_d = nc.dram_tensor("q", [B_PER_CORE, C, HW], F32, kind="ExternalInput").ap()
    s_d = nc.dram_tensor(
        "s", [B_PER_CORE, N_SUP, C, HW], F32, kind="ExternalInput"
    ).ap()
    wqk_d = nc.dram_tensor("wqk", [DK, C], F32, kind="ExternalInput").ap()
    wv_d = nc.dram_tensor("wv", [DK, C], F32, kind="ExternalInput").ap()
    out_d = nc.dram_tensor("out", [1, 1], F32, kind="ExternalOutput").ap()

    with tile.TileContext(nc) as tc:
        with (
            tc.tile_pool(name="const", bufs=1) as const,
            tc.tile_pool(name="sg", bufs=16) as sg,
            tc.tile_pool(name="kvbf", bufs=2) as kvbf,
            tc.tile_pool(name="etp", bufs=8) as etp,
            tc.tile_pool(name="vtp", bufs=10) as vtp,
            tc.tile_pool(name="small", bufs=10) as small,
            tc.tile_pool(name="ps_proj", bufs=3, space="PSUM") as ps_proj,
            tc.tile_pool(name="ps_sim", bufs=2, space="PSUM") as ps_sim,
            tc.tile_pool(name="ps_vt", bufs=2, space="PSUM") as ps_vt,
            tc.tile_pool(name="ps_ctx", bufs=1, space="PSUM") as ps_ctx,
        ):
            # ---- constants / weights ----
            id_f32 = const.tile([128, 128], F32, tag="id_f32")
            make_identity(nc, id_f32)
            id_bf = const.tile([128, 128], BF16, tag="id_bf")
            make_identity(nc, id_bf)
            ones_bf = const.tile([128, 1], BF16, tag="ones_bf")
            nc.vector.memset(ones_bf, 1.0)

            wqk_sb = const.tile([128, C], F32, tag="wqk_sb")
            nc.sync.dma_start(out=wqk_sb, in_=wqk_d)
            wv_sb = const.tile([128, C], F32, tag="wv_sb")
            nc.sync.dma_start(out=wv_sb, in_=wv_d)

            # query load (before S groups: small, needed early for qq/qv)
            qsb = []
            for cc in range(CCH):
                qt = const.tile([128, B_PER_CORE * HW], F32R, tag=f"qsb{cc}")
                src = q_d[:, cc * 128 : (cc + 1) * 128, :].rearrange(
                    "b c ij -> c b ij"
                ).bitcast(F32R)
                nc.sync.dma_start(
                    out=qt.rearrange("p (b ij) -> p b ij", b=B_PER_CORE), in_=src
                )
                qsb.append(qt)

            wqkT = []
            wvT = []
            for cc in range(CCH):
                for (src, dstl, nm) in ((wqk_sb, wqkT, "qk"), (wv_sb, wvT, "v")):
                    pt = ps_vt.tile([128, 128], F32, tag="ps_vt")
                    nc.tensor.transpose(pt, src[:, cc * 128 : (cc + 1) * 128], id_f32)
                    wt = const.tile([128, 128], F32R, tag=f"w{nm}T{cc}")
                    nc.vector.tensor_copy(wt, pt)
                    dstl.append(wt)

            # ---- query projections (all 4 batches at once) ----
            qq_bf = const.tile([128, B_PER_CORE * HW], BF16, tag="qq_bf")
            qv_sb = const.tile([128, B_PER_CORE * HW], F32, tag="qv_sb")
            for wT, dst in ((wqkT, qq_bf), (wvT, qv_sb)):
                for half in range(2):
                    hw0 = half * 392
                    pq = ps_proj.tile([128, FT], F32, tag="ps_proj")
                    for cc in range(CCH):
                        nc.tensor.matmul(
                            pq[:, :392],
                            lhsT=wT[cc],
                            rhs=qsb[cc][:, hw0 : hw0 + 392],
                            start=(cc == 0),
                            stop=(cc == CCH - 1),
                        )
                    nc.vector.tensor_copy(dst[:, hw0 : hw0 + 392], pq[:, :392])

            # qv^T per (b, hw-chunk): [hw<=128, 128] fp32 — matches ctx layout
            qvT = {}
            for b in range(B_PER_CORE):
                for h in range(2):
                    hww = 128 if h == 0 else HW - 128
                    pt = ps_vt.tile([128, 128], F32, tag="ps_vt")
                    nc.tensor.transpose(
                        pt[:hww, :],
                        qv_sb[:, b * HW + h * 128 : b * HW + h * 128 + hww],
                        id_f32,
                    )
                    qt = const.tile([128, 128], F32, tag=f"qvT{b}_{h}")
                    nc.vector.tensor_copy(qt[:hww, :], pt[:hww, :])
                    qvT[(b, h)] = qt

            partials = const.tile([128, 2 * B_PER_CORE], F32, tag="partials")
            nc.vector.memset(partials, 0.0)

            # ---- interleaved pipeline with a cross-batch attention queue ----
            # Attention work (B: V^T transpose, C: sim+exp, D: PV accum) is
            # emitted via global cursors capped per projection-half, so the
            # surplus from late-batch halves smooths into the next batch's
            # thin early halves and every engine sees a near-constant rate.
            batches = {}

            def emit_drain(b):
                bs = batches[b]
                for h in range(2):
                    hww = 128 if h == 0 else HW - 128
                    r = small.tile([128, 1], F32, tag="recip")
                    nc.vector.reciprocal(
                        r[:hww], bs["pc"][:hww, h * 256 + 128 : h * 256 + 129]
                    )
                    d = small.tile([128, 128], F32, tag="diff")
                    nc.vector.scalar_tensor_tensor(
                        out=d[:hww, :],
                        in0=bs["pc"][:hww, h * 256 : h * 256 + 128],
                        scalar=r[:hww],
                        in1=qvT[(b, h)][:hww, :],
                        op0=mybir.AluOpType.mult,
                        op1=mybir.AluOpType.subtract,
                    )
                    d2 = small.tile([128, 128], F32, tag="d2")
                    nc.vector.scalar_tensor_tensor(
                        out=d2[:hww, :],
                        in0=d[:hww, :],
                        scalar=1.0,
                        in1=d[:hww, :],
                        op0=mybir.AluOpType.mult,
                        op1=mybir.AluOpType.mult,
                        accum_out=partials[:hww, 2 * b + h : 2 * b + h + 1],
                    )

            def emit_B(bs):
                j = bs["nB"]
                cw = min(128, NIJ - j * 128)
                pt = ps_vt.tile([128, 128], BF16, tag="ps_vt")
                _lab(nc.tensor.transpose(
                    pt[:cw, :], bs["v_bf"][:, j * 128 : j * 128 + cw], id_bf
                ), f"B.tr b{bs['b']} j{j}")
                vt = vtp.tile([128, 128], BF16, tag="vt")
                _lab(nc.vector.tensor_copy(vt[:cw, :], pt[:cw, :]), f"B.cp b{bs['b']} j{j}")
                bs["vt_tiles"][j] = vt
                bs["nB"] += 1

            def emit_C(bs):
                p = bs["nC"]
                ps = ps_sim.tile([128, 392], F32, tag="ps_sim")
                solo = 2 * p + 1 >= NCH
                for s in range(1 if solo else 2):
                    j = 2 * p + s
                    cw = min(128, NIJ - j * 128)
                    _lab(nc.tensor.matmul(
                        ps[:cw, s * HW : (s + 1) * HW],
                        lhsT=bs["k_bf"][:, j * 128 : j * 128 + cw],
                        rhs=qq_bf[:, bs["b"] * HW : (bs["b"] + 1) * HW],
                        start=True,
                        stop=True,
                    ), f"C.sim b{bs['b']} j{j}")
                e = etp.tile([128, 392], BF16, tag="et")
                if solo:
                    cw = NIJ - (2 * p) * 128
                    nc.vector.memset(e, 0.0)
                    _lab(nc.scalar.activation(
                        out=e[:cw, 0:HW],
                        in_=ps[:cw, 0:HW],
                        func=mybir.ActivationFunctionType.Exp,
                    ), f"C.exp b{bs['b']} p{p}")
                else:
                    _lab(nc.scalar.activation(
                        out=e, in_=ps, func=mybir.ActivationFunctionType.Exp
                    ), f"C.exp b{bs['b']} p{p}")
                bs["et_tiles"][p] = e
                bs["nC"] += 1

            def emit_D(bs):
                j = bs["nD"]
                e = bs["et_tiles"][j // 2]
                c0 = (j % 2) * HW
                for h in range(2):
                    hww = 128 if h == 0 else HW - 128
                    lhs = e[:, c0 + h * 128 : c0 + h * 128 + hww]
                    _lab(nc.tensor.matmul(
                        bs["pc"][:hww, h * 256 : h * 256 + 128],
                        lhsT=lhs,
                        rhs=bs["vt_tiles"][j],
                        start=(j == 0),
                        stop=(j == NCH - 1),
                    ), f"D.ctx b{bs['b']} j{j} h{h}")
                    _lab(nc.tensor.matmul(
                        bs["pc"][:hww, h * 256 + 128 : h * 256 + 129],
                        lhsT=lhs,
                        rhs=ones_bf,
                        start=(j == 0),
                        stop=(j == NCH - 1),
                    ), f"D.den b{bs['b']} j{j} h{h}")
                bs["nD"] += 1
                if bs["nD"] == NCH:
                    emit_drain(bs["b"])

            # lags keep each consumer behind its producer by more than the
            # producer's copy/exp latency, so PE never head-of-line stalls
            def b_ready(bs, lag):
                cov = bs["cov"] - lag if bs["cov"] < NIJ else NIJ
                return bs["nB"] < NCH and min(128 * (bs["nB"] + 1), NIJ) <= cov

            def c_ready(bs, lag):
                cov = bs["cov"] - lag if bs["cov"] < NIJ else NIJ
                return bs["nC"] < NPAIR and min(256 * (bs["nC"] + 1), NIJ) <= cov

            def d_ready(bs, lag):
                return bs["nD"] < min(min(2 * bs["nC"], NCH), bs["nB"])

            def emit_attn(capB, capC, capD, lag=FT):
                for cap, ready, emit in (
                    (capB, b_ready, emit_B),
                    (capC, c_ready, emit_C),
                    (capD, d_ready, emit_D),
                ):
                    done = 0
                    for bs in [batches[i] for i in sorted(batches)]:
                        while done < cap and ready(bs, lag):
                            emit(bs)
                            done += 1

            for b in range(B_PER_CORE):
                k_bf = kvbf.tile([128, NIJ], BF16, tag="k_bf")
                v_bf = kvbf.tile([128, NIJ], BF16, tag="v_bf")
                # both h ctx+den accumulators, one bank-aligned PSUM tile:
                # cols [h*256, h*256+128) = ctx, col h*256+128 = denom
                pc = ps_ctx.tile([128, 512], F32, tag="ps_ctx")
                batches[b] = {
                    "b": b,
                    "k_bf": k_bf,
                    "v_bf": v_bf,
                    "pc": pc,
                    "vt_tiles": [None] * NCH,
                    "et_tiles": [None] * NPAIR,
                    "nB": 0, "nC": 0, "nD": 0, "cov": 0,
                }
                bs = batches[b]
                for g in range(NG):
                    sgt = []
                    for cc in range(CCH):
                        s_t = sg.tile([128, GW], F32R, tag="s_t")
                        src = s_d[
                            b, g * GN : (g + 1) * GN,
                            cc * 128 : (cc + 1) * 128, :,
                        ].rearrange("n c ij -> c n ij").bitcast(F32R)
                        _lab(nc.sync.dma_start(
                            out=s_t.rearrange("p (n ij) -> p n ij", n=GN),
                            in_=src,
                        ), f"A.dma b{b} g{g} cc{cc}")
                        sgt.append(s_t)
                    for half in range(2):
                        c0 = g * GW + half * FT
                        pk = ps_proj.tile([128, FT], F32, tag="ps_proj")
                        for cc in range(CCH):
                            _lab(nc.tensor.matmul(
                                pk,
                                lhsT=wqkT[cc],
                                rhs=sgt[cc][:, half * FT : (half + 1) * FT],
                                start=(cc == 0),
                                stop=(cc == CCH - 1),
                            ), f"A.k b{b} g{g} h{half} cc{cc}")
                        _lab(nc.vector.tensor_copy(bs["k_bf"][:, c0 : c0 + FT], pk), f"A.kcp b{b} g{g} h{half}")
                        pv = ps_proj.tile([128, FT], F32, tag="ps_proj")
                        for cc in range(CCH):
                            _lab(nc.tensor.matmul(
                                pv,
                                lhsT=wvT[cc],
                                rhs=sgt[cc][:, half * FT : (half + 1) * FT],
                                start=(cc == 0),
                                stop=(cc == CCH - 1),
                            ), f"A.v b{b} g{g} h{half} cc{cc}")
                        _lab(nc.scalar.copy(bs["v_bf"][:, c0 : c0 + FT], pv), f"A.vcp b{b} g{g} h{half}")
                        bs["cov"] = c0 + FT
                        if b == 0 and g == 0 and half == 1:
                            emit_query_section()
                        emit_attn(99, 99, 99)

                emit_attn(999, 999, 999, lag=0)

            # flush all remaining attention work (tail)
            emit_attn(999, 999, 999, lag=0)

            # ---- final reduction to scalar ----
            tot = small.tile([128, 1], F32, tag="tot")
            nc.vector.reduce_sum(tot, partials, axis=mybir.AxisListType.X)
            ones = small.tile([128, 1], F32, tag="ones")
            nc.vector.memset(ones, 1.0)
            pf = ps_vt.tile([128, 128], F32, tag="ps_vt")
            nc.tensor.matmul(pf[0:1, 0:1], lhsT=tot, rhs=ones, start=True, stop=True)
            ob = small.tile([1, 1], F32, tag="ob")
            nc.vector.tensor_copy(ob, pf[0:1, 0:1])
            nc.sync.dma_start(out=out_d, in_=ob)

    nc.compile()
    return nc


_NC = None


def kernel(query_repr, supports_repr, W_qk, W_v):
    global _NC
    q = np.ascontiguousarray(np.asarray(query_repr, dtype=np.float32)).reshape(
        32, C, HW
    )
    s = np.ascontiguousarray(np.asarray(supports_repr, dtype=np.float32)).reshape(
        32, N_SUP, C, HW
    )
    wqk = np.ascontiguousarray(np.asarray(W_qk, dtype=np.float32))
    wv = np.ascontiguousarray(np.asarray(W_v, dtype=np.float32))

    if _NC is None:
        _NC = build_bass()

    in_maps = []
    for core in range(8):
        b0 = core * B_PER_CORE
        in_maps.append(
            {
                "q": np.ascontiguousarray(q[b0 : b0 + B_PER_CORE]),
                "s": np.ascontiguousarray(s[b0 : b0 + B_PER_CORE]),
                "wqk": wqk,
                "wv": wv,
            }
        )
    res = run_bass_kernel_spmd(
        _NC, in_maps, core_ids=list(range(8)),
        trace=bool(int(os.environ.get("KTRACE", "0"))),
    )
    total = sum(float(r["out"][0, 0]) for r in res.results) / float(HW)
    kernel._last_results = res
    return np.asarray(total, dtype=np.float32)


# revision 29
# speedup vs baseline: 1.0070x; 1.0070x over previous
"""CrossTransformer kernel for Trainium2, data-parallel over batch across 8 cores.

Math per batch b (B=32, N=25, C=512, H=W=14, DK=DV=128):
  qq = Wqk @ Q    [128, 196]      qv = Wv @ Q     [128, 196]
  K  = Wqk @ S    [128, 4900]     V  = Wv @ S     [128, 4900]
  simT[nij, hw] = K^T @ qq        (computed directly in transposed layout)
  E = exp(simT)                   (no max subtraction; |sim| <~ 60 is safe in fp32)
  ctx_raw[hw, dv] = sum_nij E[nij, hw]^T @ V^T;  den[hw] = sum_nij E[nij, hw]
  ctx = ctx_raw / den
  partial += sum((qv^T - ctx)^2)
Output per core: scalar partial sum over its 4 batches; host sums and divides by H*W.

Pipeline: S streams in n-aligned [128, 980] fp32 DMA groups (784B descriptors,
the DMA-bandwidth floor). Attention work (V transpose, sim, exp, PV accumulate)
is emitted chunk-by-chunk as soon as projection tiles cover it, so every engine
runs concurrently with the DMA stream and the post-DMA tail is tiny.
"""

import os
import sys

sys.path.insert(0, "/opt/trn_rl_repo")

import numpy as np

ILABELS = {}


def _lab(inst, label):
    try:
        ILABELS[inst.ins.name] = label
    except Exception:
        pass
    return inst

import concourse.bass as bass
import concourse.bacc as bacc
import concourse.mybir as mybir
import concourse.tile as tile
from concourse.bass_utils import run_bass_kernel_spmd
from concourse.masks import make_identity

F32 = mybir.dt.float32
F32R = mybir.dt.float32r
BF16 = mybir.dt.bfloat16

B_PER_CORE = 4
N_SUP = 25
C = 512
HW = 196
NIJ = N_SUP * HW  # 4900
DK = 128
CCH = C // 128            # 4 c-chunks
GN = 5                    # support images per DMA group
GW = GN * HW              # 980 nij per group
NG = NIJ // GW            # 5 groups per batch
FT = 490                  # matmul tile width (2 per group, fits one PSUM bank)
NCH = (NIJ + 127) // 128  # 39 nij chunks of <=128
NPAIR = (NCH + 1) // 2    # 20 sim/exp pairs (19 full + 1 solo)


def build_bass():
    nc = bacc.Bacc(
        "TRN2", target_bir_lowering=False, debug=False, enable_asserts=False
    )
    q_d = nc.dram_tensor("q", [B_PER_CORE, C, HW], F32, kind="ExternalInput").ap()
    s_d = nc.dram_tensor(
        "s", [B_PER_CORE, N_SUP, C, HW], F32, kind="ExternalInput"
    ).ap()
    wqk_d = nc.dram_tensor("wqk", [DK, C], F32, kind="ExternalInput").ap()
    wv_d = nc.dram_tensor("wv", [DK, C], F32, kind="ExternalInput").ap()
    out_d = nc.dram_tensor("out", [1, 1], F32, kind="ExternalOutput").ap()

    with tile.TileContext(nc) as tc:
        with (
            tc.tile_pool(name="const", bufs=1) as const,
            tc.tile_pool(name="sg", bufs=16) as sg,
            tc.tile_pool(name="kvbf", bufs=2) as kvbf,
            tc.tile_pool(name="etp", bufs=8) as etp,
            tc.tile_pool(name="vtp", bufs=10) as vtp,
            tc.tile_pool(name="small", bufs=10) as small,
            tc.tile_pool(name="ps_proj", bufs=3, space="PSUM") as ps_proj,
            tc.tile_pool(name="ps_sim", bufs=2, space="PSUM") as ps_sim,
            tc.tile_pool(name="ps_vt", bufs=2, space="PSUM") as ps_vt,
            tc.tile_pool(name="ps_ctx", bufs=1, space="PSUM") as ps_ctx,
        ):
            # ---- constants / weights ----
            id_f32 = const.tile([128, 128], F32, tag="id_f32")
            make_identity(nc, id_f32)
            id_bf = const.tile([128, 128], BF16, tag="id_bf")
            make_identity(nc, id_bf)
            ones_bf = const.tile([128, 1], BF16, tag="ones_bf")
            nc.vector.memset(ones_bf, 1.0)

            wqk_sb = const.tile([128, C], F32, tag="wqk_sb")
            nc.sync.dma_start(out=wqk_sb, in_=wqk_d)
            wv_sb = const.tile([128, C], F32, tag="wv_sb")
            nc.sync.dma_start(out=wv_sb, in_=wv_d)

            # query load (before S groups: small, needed early for qq/qv)
            qsb = []
            for cc in range(CCH):
                qt = const.tile([128, B_PER_CORE * HW], F32R, tag=f"qsb{cc}")
                src = q_d[:, cc * 128 : (cc + 1) * 128, :].rearrange(
                    "b c ij -> c b ij"
                ).bitcast(F32R)
                nc.sync.dma_start(
                    out=qt.rearrange("p (b ij) -> p b ij", b=B_PER_CORE), in_=src
                )
                qsb.append(qt)

            wqkT = []
            wvT = []
            for cc in range(CCH):
                for (src, dstl, nm) in ((wqk_sb, wqkT, "qk"), (wv_sb, wvT, "v")):
                    pt = ps_vt.tile([128, 128], F32, tag="ps_vt")
                    nc.tensor.transpose(pt, src[:, cc * 128 : (cc + 1) * 128], id_f32)
                    wt = const.tile([128, 128], F32R, tag=f"w{nm}T{cc}")
                    nc.vector.tensor_copy(wt, pt)
                    dstl.append(wt)

            # ---- query projections (all 4 batches at once) ----
            qq_bf = const.tile([128, B_PER_CORE * HW], BF16, tag="qq_bf")
            qv_sb = const.tile([128, B_PER_CORE * HW], F32, tag="qv_sb")
            for wT, dst in ((wqkT, qq_bf), (wvT, qv_sb)):
                for half in range(2):
                    hw0 = half * 392
                    pq = ps_proj.tile([128, FT], F32, tag="ps_proj")
                    for cc in range(CCH):
                        nc.tensor.matmul(
                            pq[:, :392],
                            lhsT=wT[cc],
                            rhs=qsb[cc][:, hw0 : hw0 + 392],
                            start=(cc == 0),
                            stop=(cc == CCH - 1),
                        )
                    nc.vector.tensor_copy(dst[:, hw0 : hw0 + 392], pq[:, :392])

            # qv^T per (b, hw-chunk): [hw<=128, 128] fp32 — matches ctx layout
            qvT = {}
            for b in range(B_PER_CORE):
                for h in range(2):
                    hww = 128 if h == 0 else HW - 128
                    pt = ps_vt.tile([128, 128], F32, tag="ps_vt")
                    nc.tensor.transpose(
                        pt[:hww, :],
                        qv_sb[:, b * HW + h * 128 : b * HW + h * 128 + hww],
                        id_f32,
                    )
                    qt = const.tile([128, 128], F32, tag=f"qvT{b}_{h}")
                    nc.vector.tensor_copy(qt[:hww, :], pt[:hww, :])
                    qvT[(b, h)] = qt

            partials = const.tile([128, 2 * B_PER_CORE], F32, tag="partials")
            nc.vector.memset(partials, 0.0)
            emit_query_section()

            # ---- interleaved pipeline with a cross-batch attention queue ----
            # Attention work (B: V^T transpose, C: sim+exp, D: PV accum) is
            # emitted via global cursors capped per projection-half, so the
            # surplus from late-batch halves smooths into the next batch's
            # thin early halves and every engine sees a near-constant rate.
            batches = {}

            def emit_drain(b):
                bs = batches[b]
                for h in range(2):
                    hww = 128 if h == 0 else HW - 128
                    r = small.tile([128, 1], F32, tag="recip")
                    nc.vector.reciprocal(
                        r[:hww], bs["pc"][:hww, h * 256 + 128 : h * 256 + 129]
                    )
                    d = small.tile([128, 128], F32, tag="diff")
                    nc.vector.scalar_tensor_tensor(
                        out=d[:hww, :],
                        in0=bs["pc"][:hww, h * 256 : h * 256 + 128],
                        scalar=r[:hww],
                        in1=qvT[(b, h)][:hww, :],
                        op0=mybir.AluOpType.mult,
                        op1=mybir.AluOpType.subtract,
                    )
                    d2 = small.tile([128, 128], F32, tag="d2")
                    nc.vector.scalar_tensor_tensor(
                        out=d2[:hww, :],
                        in0=d[:hww, :],
                        scalar=1.0,
                        in1=d[:hww, :],
                        op0=mybir.AluOpType.mult,
                        op1=mybir.AluOpType.mult,
                        accum_out=partials[:hww, 2 * b + h : 2 * b + h + 1],
                    )

            def emit_B(bs):
                j = bs["nB"]
                cw = min(128, NIJ - j * 128)
                pt = ps_vt.tile([128, 128], BF16, tag="ps_vt")
                _lab(nc.tensor.transpose(
                    pt[:cw, :], bs["v_bf"][:, j * 128 : j * 128 + cw], id_bf
                ), f"B.tr b{bs['b']} j{j}")
                vt = vtp.tile([128, 128], BF16, tag="vt")
                _lab(nc.vector.tensor_copy(vt[:cw, :], pt[:cw, :]), f"B.cp b{bs['b']} j{j}")
                bs["vt_tiles"][j] = vt
                bs["nB"] += 1

            def emit_C(bs):
                p = bs["nC"]
                ps = ps_sim.tile([128, 392], F32, tag="ps_sim")
                solo = 2 * p + 1 >= NCH
                for s in range(1 if solo else 2):
                    j = 2 * p + s
                    cw = min(128, NIJ - j * 128)
                    _lab(nc.tensor.matmul(
                        ps[:cw, s * HW : (s + 1) * HW],
                        lhsT=bs["k_bf"][:, j * 128 : j * 128 + cw],
                        rhs=qq_bf[:, bs["b"] * HW : (bs["b"] + 1) * HW],
                        start=True,
                        stop=True,
                    ), f"C.sim b{bs['b']} j{j}")
                e = etp.tile([128, 392], BF16, tag="et")
                if solo:
                    cw = NIJ - (2 * p) * 128
                    nc.vector.memset(e, 0.0)
                    _lab(nc.scalar.activation(
                        out=e[:cw, 0:HW],
                        in_=ps[:cw, 0:HW],
                        func=mybir.ActivationFunctionType.Exp,
                    ), f"C.exp b{bs['b']} p{p}")
                else:
                    _lab(nc.scalar.activation(
                        out=e, in_=ps, func=mybir.ActivationFunctionType.Exp
                    ), f"C.exp b{bs['b']} p{p}")
                bs["et_tiles"][p] = e
                bs["nC"] += 1

            def emit_D(bs):
                j = bs["nD"]
                e = bs["et_tiles"][j // 2]
                c0 = (j % 2) * HW
                for h in range(2):
                    hww = 128 if h == 0 else HW - 128
                    lhs = e[:, c0 + h * 128 : c0 + h * 128 + hww]
                    _lab(nc.tensor.matmul(
                        bs["pc"][:hww, h * 256 : h * 256 + 128],
                        lhsT=lhs,
                        rhs=bs["vt_tiles"][j],
                        start=(j == 0),
                        stop=(j == NCH - 1),
                    ), f"D.ctx b{bs['b']} j{j} h{h}")
                    _lab(nc.tensor.matmul(
                        bs["pc"][:hww, h * 256 + 128 : h * 256 + 129],
                        lhsT=lhs,
                        rhs=ones_bf,
                        start=(j == 0),
                        stop=(j == NCH - 1),
                    ), f"D.den b{bs['b']} j{j} h{h}")
                bs["nD"] += 1
                if bs["nD"] == NCH:
                    emit_drain(bs["b"])

            # lags keep each consumer behind its producer by more than the
            # producer's copy/exp latency, so PE never head-of-line stalls
            def b_ready(bs, lag):
                cov = bs["cov"] - lag if bs["cov"] < NIJ else NIJ
                return bs["nB"] < NCH and min(128 * (bs["nB"] + 1), NIJ) <= cov

            def c_ready(bs, lag):
                cov = bs["cov"] - lag if bs["cov"] < NIJ else NIJ
                return bs["nC"] < NPAIR and min(256 * (bs["nC"] + 1), NIJ) <= cov

            def d_ready(bs, lag):
                return bs["nD"] < min(min(2 * bs["nC"], NCH), bs["nB"])

            def emit_attn(capB, capC, capD, lag=FT):
                for cap, ready, emit in (
                    (capB, b_ready, emit_B),
                    (capC, c_ready, emit_C),
                    (capD, d_ready, emit_D),
                ):
                    done = 0
                    for bs in [batches[i] for i in sorted(batches)]:
                        while done < cap and ready(bs, lag):
                            emit(bs)
                            done += 1

            for b in range(B_PER_CORE):
                k_bf = kvbf.tile([128, NIJ], BF16, tag="k_bf")
                v_bf = kvbf.tile([128, NIJ], BF16, tag="v_bf")
                # both h ctx+den accumulators, one bank-aligned PSUM tile:
                # cols [h*256, h*256+128) = ctx, col h*256+128 = denom
                pc = ps_ctx.tile([128, 512], F32, tag="ps_ctx")
                batches[b] = {
                    "b": b,
                    "k_bf": k_bf,
                    "v_bf": v_bf,
                    "pc": pc,
                    "vt_tiles": [None] * NCH,
                    "et_tiles": [None] * NPAIR,
                    "nB": 0, "nC": 0, "nD": 0, "cov": 0,
                }
                bs = batches[b]
                for g in range(NG):
                    sgt = []
                    for cc in range(CCH):
                        s_t = sg.tile([128, GW], F32R, tag="s_t")
                        src = s_d[
                            b, g * GN : (g + 1) * GN,
                            cc * 128 : (cc + 1) * 128, :,
                        ].rearrange("n c ij -> c n ij").bitcast(F32R)
                        _lab(nc.sync.dma_start(
                            out=s_t.rearrange("p (n ij) -> p n ij", n=GN),
                            in_=src,
                        ), f"A.dma b{b} g{g} cc{cc}")
                        sgt.append(s_t)
                    for half in range(2):
                        c0 = g * GW + half * FT
                        pk = ps_proj.tile([128, FT], F32, tag="ps_proj")
                        for cc in range(CCH):
                            _lab(nc.tensor.matmul(
                                pk,
                                lhsT=wqkT[cc],
                                rhs=sgt[cc][:, half * FT : (half + 1) * FT],
                                start=(cc == 0),
                                stop=(cc == CCH - 1),
                            ), f"A.k b{b} g{g} h{half} cc{cc}")
                        _lab(nc.vector.tensor_copy(bs["k_bf"][:, c0 : c0 + FT], pk), f"A.kcp b{b} g{g} h{half}")
                        pv = ps_proj.tile([128, FT], F32, tag="ps_proj")
                        for cc in range(CCH):
                            _lab(nc.tensor.matmul(
                                pv,
                                lhsT=wvT[cc],
                                rhs=sgt[cc][:, half * FT : (half + 1) * FT],
                                start=(cc == 0),
                                stop=(cc == CCH - 1),
                            ), f"A.v b{b} g{g} h{half} cc{cc}")
                        _lab(nc.scalar.copy(bs["v_bf"][:, c0 : c0 + FT], pv), f"A.vcp b{b} g{g} h{half}")
                        bs["cov"] = c0 + FT
                        emit_attn(99, 99, 99)

                emit_attn(999, 999, 999, lag=0)

            # flush all remaining attention work (tail)
            emit_attn(999, 999, 999, lag=0)

            # ---- final reduction to scalar ----
            tot = small.tile([128, 1], F32, tag="tot")
            nc.vector.reduce_sum(tot, partials, axis=mybir.AxisListType.X)
            ones = small.tile([128, 1], F32, tag="ones")
            nc.vector.memset(ones, 1.0)
            pf = ps_vt.tile([128, 128], F32, tag="ps_vt")
            nc.tensor.matmul(pf[0:1, 0:1], lhsT=tot, rhs=ones, start=True, stop=True)
            ob = small.tile([1, 1], F32, tag="ob")
            nc.vector.tensor_copy(ob, pf[0:1, 0:1])
            nc.sync.dma_start(out=out_d, in_=ob)

    nc.compile()
    return nc


_NC = None


def kernel(query_repr, supports_repr, W_qk, W_v):
    global _NC
    q = np.ascontiguousarray(np.asarray(query_repr, dtype=np.float32)).reshape(
        32, C, HW
    )
    s = np.ascontiguousarray(np.asarray(supports_repr, dtype=np.float32)).reshape(
        32, N_SUP, C, HW
    )
    wqk = np.ascontiguousarray(np.asarray(W_qk, dtype=np.float32))
    wv = np.ascontiguousarray(np.asarray(W_v, dtype=np.float32))

    if _NC is None:
        _NC = build_bass()

    in_maps = []
    for core in range(8):
        b0 = core * B_PER_CORE
        in_maps.append(
            {
                "q": np.ascontiguousarray(q[b0 : b0 + B_PER_CORE]),
                "s": np.ascontiguousarray(s[b0 : b0 + B_PER_CORE]),
                "wqk": wqk,
                "wv": wv,
            }
        )
    res = run_bass_kernel_spmd(
        _NC, in_maps, core_ids=list(range(8)),
        trace=bool(int(os.environ.get("KTRACE", "0"))),
    )
    total = sum(float(r["out"][0, 0]) for r in res.results) / float(HW)
    kernel._last_results = res
    return np.asarray(total, dtype=np.float32)
